# revision 14
# baseline (speedup 1.0000x reference)
"""Trainium2 Bass kernel for im2col Conv2d dot-product:
out[b, n] = <enc_x[b, n, :], w_flat> + bias.

Data-parallel over batch: 8 batches per NeuronCore x 8 cores.
Per core: x is [401408, 49] -> out [401408] fp32.

v5: dense phase-packed TensorE matmul.

DMA on TRN2 runs at full rate (~427 GB/s, 27 GB/s x 16 engines)
ONLY for 128-partition tiles (measured: 98 partitions -> ~260 GB/s,
112 -> ~263, 120 -> ~199). So the moving operand must fill all 128
partitions with real data, but windows are 49 long: instead of
2-windows-per-column (98 rows + 30 pad), pack the im2col stream
DENSELY: column c of "phase" phi holds flat element 128*phi + c of a
6272-element group (= 128 windows x 49 = lcm(49,128) structure).
The identity 128*phi + c = 49*m + k uniquely assigns every (phi, c)
to window-row m, element k, so 49 banded stationaries
S_phi[c, m] = w[128*phi + c - 49*m] (in [0,49)) make the 49
phase-matmuls accumulate exact per-window dot products into
PSUM[m, g] with zero padding and zero redundancy.

Stream order puts window m*3136+g at stream slot 128*g+m, so
PSUM[m, g-chunk] lands in natural window order: out DMA is
contiguous per partition.  Each 448-column block closes after its
49 matmuls -> ScalarE bias-add -> out DMA, fully pipelined (no
end-of-kernel PSUM flush tail).

Per core: DMA in 39.3 MB bf16 (~95-105 us at full rate), TensorE
153664 moving columns ~ 64 us @2.4 GHz, ScalarE 3.6 us, DVE idle.
"""

from contextlib import ExitStack

import numpy as np
import ml_dtypes

import concourse.bass as bass
import concourse.tile as tile
from concourse import mybir

B = 64
WINDOWS = 50176
K = 49
NCORES = 8
BPC = B // NCORES            # batches per core
NWIN = BPC * WINDOWS         # 401408 windows per core
ROWS = 128                   # window-row grid: window = m*J + g
J = NWIN // ROWS             # 3136
NPHI = K                     # 49 phases
GW = 224                     # g-columns per block (PSUM region, 14*GW = J)
NBLK = J // GW               # 14

FP32 = mybir.dt.float32
BF16 = mybir.dt.bfloat16
BF16_NP = ml_dtypes.bfloat16

_NC = None


def _build_nc():
    nc = bass.Bass(trn_type="TRN2", debug=False, num_devices=NCORES)

    # x[blk, c, NPHI*GW]: tile column NPHI*gw? no: column phi*GW+gw
    x = nc.dram_tensor(
        "x", [NBLK, ROWS, NPHI * GW], BF16, kind="ExternalInput"
    ).ap()
    s = nc.dram_tensor("s", [ROWS, NPHI * ROWS], BF16, kind="ExternalInput").ap()
    b = nc.dram_tensor("b", [1], FP32, kind="ExternalInput").ap()
    out = nc.dram_tensor("out", [NWIN], FP32, kind="ExternalOutput").ap()

    with tile.TileContext(nc) as tc, ExitStack() as ctx:
        consts = ctx.enter_context(tc.tile_pool(name="consts", bufs=1))
        xpool = ctx.enter_context(tc.tile_pool(name="x", bufs=3))
        opool = ctx.enter_context(tc.tile_pool(name="o", bufs=2))
        ppool = ctx.enter_context(tc.tile_pool(name="psum", bufs=4, space="PSUM"))

        stat = consts.tile([ROWS, NPHI * ROWS], BF16)
        nc.scalar.dma_start(out=stat[:], in_=s)
        bb = consts.tile([ROWS, 1], FP32)
        nc.scalar.dma_start(
            out=bb[:],
            in_=bass.AP(tensor=b.tensor, offset=b.offset, ap=[[0, ROWS]] + list(b.ap)),
        )

        for blk in range(NBLK):
            xt = xpool.tile([ROWS, NPHI * GW], BF16, tag="xt", name=f"xt{blk}")
            src = bass.AP(
                tensor=x.tensor,
                offset=x.offset + blk * ROWS * NPHI * GW,
                ap=[[NPHI * GW, ROWS], [1, NPHI * GW]],
            )
            nc.sync.dma_start(out=xt[:], in_=src)

            # allocate a full PSUM bank (512 fp32) for alignment; use GW cols
            acc = ppool.tile([ROWS, 512], FP32, tag="acc", name=f"acc{blk}")
            for phi in range(NPHI):
                nc.tensor.matmul(
                    acc[:, 0:GW],
                    stat[:, phi * ROWS : (phi + 1) * ROWS],
                    xt[:, phi * GW : (phi + 1) * GW],
                    start=(phi == 0),
                    stop=(phi == NPHI - 1),
                )

            ot = opool.tile([ROWS, GW], FP32, tag="ot", name=f"ot{blk}")
            nc.scalar.activation(
                out=ot[:],
                in_=acc[:, 0:GW],
                func=mybir.ActivationFunctionType.Identity,
                bias=bb[:, 0:1],
                scale=1.0,
            )
            dst = bass.AP(
                tensor=out.tensor,
                offset=out.offset + blk * GW,
                ap=[[J, ROWS], [1, GW]],
            )
            # last block rides the (by-then idle) HWDGE sync queue
            oeng = nc.sync if blk == NBLK - 1 else nc.gpsimd
            oeng.dma_start(out=dst, in_=ot[:])

    return nc


def _split_ctrl_waits(nc, max_waits=1):
    """Work around a walrus codegen limit on this build: instructions accept
    only one sync-wait command. Hoist extra waits onto dedicated no-op
    instructions inserted just before, preserving per-engine order."""
    from concourse import mybir

    for f in nc.m.functions:
        for blk in f.blocks:
            insts = blk.instructions
            i = 0
            while i < len(insts):
                ins = insts[i]
                if (
                    ins.sync_info is not None
                    and len(ins.sync_info.on_wait) > max_waits
                ):
                    waits = list(ins.sync_info.on_wait)
                    keep, extra = waits[:max_waits], waits[max_waits:]
                    ins.sync_info.on_wait = keep
                    for j, wchunk in enumerate(extra):
                        nop = mybir.InstNoOp(
                            name=f"{ins.name}-wsplit{j}",
                            sync_info=mybir.SyncInfo(on_wait=[wchunk], on_update=[]),
                            bass_nofuse=True,
                            engine=ins.engine,
                        )
                        nc.register_instruction(nop, overwrite=True)
                        insts.insert(i, nop)
                        i += 1
                i += 1


def _get_nc():
    global _NC
    if _NC is None:
        _NC = _build_nc()
        _split_ctrl_waits(_NC)
    return _NC


# z = 128*phi + c = 49*m + k for z in [0, 6272)
_Z = np.arange(ROWS * K)
_MZ = _Z // K
_KZ = _Z % K


def _pack_inputs(enc_x, weight, bias):
    """Host-side repack: dense phase-packed bf16 layout + banded stationaries."""
    # xr[m, g, k] = enc_x_core[m*J + g, k]; phase view:
    # X4[phi, c, g] = xr[mz, g, kz] at z = 128*phi + c
    xr = np.asarray(enc_x, dtype=np.float32).reshape(NCORES, ROWS, J, K)
    xb = xr.astype(BF16_NP)
    xT = np.empty((NCORES, NBLK, ROWS, NPHI * GW), dtype=BF16_NP)
    for cix in range(NCORES):
        g1 = xb[cix][_MZ, :, _KZ]                      # [6272, J]
        x4 = g1.reshape(NPHI, ROWS, NBLK, GW)          # [phi, c, blk, gw]
        xT[cix] = x4.transpose(2, 1, 0, 3).reshape(NBLK, ROWS, NPHI * GW)

    wb = np.asarray(weight, dtype=np.float32).reshape(K).astype(BF16_NP)
    stat = np.zeros((ROWS, NPHI * ROWS), dtype=BF16_NP)
    for phi in range(NPHI):
        z = 128 * phi + np.arange(ROWS)                # z for each c
        m = z // K
        k = z % K
        valid = m < ROWS
        stat[np.arange(ROWS)[valid], phi * ROWS + m[valid]] = wb[k[valid]]

    bf = np.ascontiguousarray(np.asarray(bias), dtype=np.float32).reshape(1)
    return xT, stat, bf


def run(enc_x, weight, bias, trace=False, **spmd_kwargs):
    """Run on 8 NeuronCores; returns (out [B, WINDOWS] fp32, BassKernelResults)."""
    from concourse.bass_utils import run_bass_kernel_spmd

    nc = _get_nc()
    xT, stat, bf = _pack_inputs(enc_x, weight, bias)
    in_maps = [{"x": xT[i], "s": stat, "b": bf} for i in range(NCORES)]
    res = run_bass_kernel_spmd(
        nc, in_maps, list(range(NCORES)), trace=trace, **spmd_kwargs
    )
    out = np.stack([res.results[i]["out"] for i in range(NCORES)], axis=0)
    return out.reshape(B, WINDOWS), res


def kernel(enc_x, weight, bias, windows_nb=None):
    out, _ = run(enc_x, weight, bias)
    return out


# revision 18
# speedup vs baseline: 1.1496x; 1.1496x over previous
"""Trainium2 Bass kernel for im2col Conv2d dot-product:
out[b, n] = <enc_x[b, n, :], w_flat> + bias.

Data-parallel over batch: 8 batches per NeuronCore x 8 cores.
Per core: x is [401408, 49] -> out [401408] fp32.

v5: dense phase-packed TensorE matmul.

DMA on TRN2 runs at full rate (~427 GB/s, 27 GB/s x 16 engines)
ONLY for 128-partition tiles (measured: 98 partitions -> ~260 GB/s,
112 -> ~263, 120 -> ~199). So the moving operand must fill all 128
partitions with real data, but windows are 49 long: instead of
2-windows-per-column (98 rows + 30 pad), pack the im2col stream
DENSELY: column c of "phase" phi holds flat element 128*phi + c of a
6272-element group (= 128 windows x 49 = lcm(49,128) structure).
The identity 128*phi + c = 49*m + k uniquely assigns every (phi, c)
to window-row m, element k, so 49 banded stationaries
S_phi[c, m] = w[128*phi + c - 49*m] (in [0,49)) make the 49
phase-matmuls accumulate exact per-window dot products into
PSUM[m, g] with zero padding and zero redundancy.

Stream order puts window m*3136+g at stream slot 128*g+m, so
PSUM[m, g-chunk] lands in natural window order: out DMA is
contiguous per partition.  Each 448-column block closes after its
49 matmuls -> ScalarE bias-add -> out DMA, fully pipelined (no
end-of-kernel PSUM flush tail).

Per core: DMA in 39.3 MB bf16 (~95-105 us at full rate), TensorE
153664 moving columns ~ 64 us @2.4 GHz, ScalarE 3.6 us, DVE idle.
"""

from contextlib import ExitStack

import numpy as np
import ml_dtypes

import concourse.bass as bass
import concourse.tile as tile
from concourse import mybir

B = 64
WINDOWS = 50176
K = 49
NCORES = 8
BPC = B // NCORES            # batches per core
NWIN = BPC * WINDOWS         # 401408 windows per core
ROWS = 128                   # window-row grid: window = m*J + g
J = NWIN // ROWS             # 3136
NPHI = K                     # 49 phases
GW = 448                     # g-columns per full block
# last full block split in two: shorter tail after the final x-DMA
BLOCKS = [448] * 6 + [224, 224]          # g-cols per block, sum = J
BATCHES = [(0, 3), (3, 3), (6, 2)]       # out-DMA batching (start, nblocks)
NBLK = len(BLOCKS)

FP32 = mybir.dt.float32
BF16 = mybir.dt.bfloat16
BF16_NP = ml_dtypes.bfloat16

_NC = None


def _build_nc():
    nc = bass.Bass(trn_type="TRN2", debug=False, num_devices=NCORES)

    # x: concatenated per-block [ROWS, NPHI*gw_b] slabs (column phi*gw_b+gw)
    x = nc.dram_tensor("x", [ROWS * NPHI * J], BF16, kind="ExternalInput").ap()
    s = nc.dram_tensor("s", [ROWS, NPHI * ROWS], BF16, kind="ExternalInput").ap()
    b = nc.dram_tensor("b", [1], FP32, kind="ExternalInput").ap()
    out = nc.dram_tensor("out", [NWIN], FP32, kind="ExternalOutput").ap()

    with tile.TileContext(nc) as tc, ExitStack() as ctx:
        consts = ctx.enter_context(tc.tile_pool(name="consts", bufs=1))
        xpool = ctx.enter_context(tc.tile_pool(name="x", bufs=3))
        opool = ctx.enter_context(tc.tile_pool(name="o", bufs=2))
        ppool = ctx.enter_context(tc.tile_pool(name="psum", bufs=4, space="PSUM"))

        stat = consts.tile([ROWS, NPHI * ROWS], BF16)
        nc.gpsimd.dma_start(out=stat[:], in_=s)
        bb = consts.tile([ROWS, 1], FP32)
        nc.gpsimd.dma_start(
            out=bb[:],
            in_=bass.AP(tensor=b.tensor, offset=b.offset, ap=[[0, ROWS]] + list(b.ap)),
        )

        blk_g0 = [sum(BLOCKS[:i]) for i in range(NBLK)]   # g-offset per block
        xoff = 0
        for bstart, bn in BATCHES:
            bcols = sum(BLOCKS[bstart : bstart + bn])
            obuf = opool.tile([ROWS, bcols], FP32, tag="ot", name=f"ot{bstart}")
            ocol = 0
            for blk in range(bstart, bstart + bn):
                gw = BLOCKS[blk]
                xt = xpool.tile([ROWS, NPHI * gw], BF16, tag="xt", name=f"xt{blk}")
                src = bass.AP(
                    tensor=x.tensor,
                    offset=x.offset + xoff,
                    ap=[[NPHI * gw, ROWS], [1, NPHI * gw]],
                )
                xoff += ROWS * NPHI * gw
                nc.sync.dma_start(out=xt[:], in_=src)

                # full PSUM bank (512 fp32) for alignment; use gw cols
                acc = ppool.tile([ROWS, 512], FP32, tag="acc", name=f"acc{blk}")
                for phi in range(NPHI):
                    nc.tensor.matmul(
                        acc[:, 0:gw],
                        stat[:, phi * ROWS : (phi + 1) * ROWS],
                        xt[:, phi * gw : (phi + 1) * gw],
                        start=(phi == 0),
                        stop=(phi == NPHI - 1),
                    )

                nc.scalar.activation(
                    out=obuf[:, ocol : ocol + gw],
                    in_=acc[:, 0:gw],
                    func=mybir.ActivationFunctionType.Identity,
                    bias=bb[:, 0:1],
                    scale=1.0,
                )
                ocol += gw

            dst = bass.AP(
                tensor=out.tensor,
                offset=out.offset + blk_g0[bstart],
                ap=[[J, ROWS], [1, bcols]],
            )
            # final batch rides the (by-then idle) HWDGE sync queue
            oeng = nc.sync if bstart + bn == NBLK else nc.gpsimd
            oeng.dma_start(out=dst, in_=obuf[:])

    return nc


def _split_ctrl_waits(nc, max_waits=1):
    """Work around a walrus codegen limit on this build: instructions accept
    only one sync-wait command. Hoist extra waits onto dedicated no-op
    instructions inserted just before, preserving per-engine order."""
    from concourse import mybir

    for f in nc.m.functions:
        for blk in f.blocks:
            insts = blk.instructions
            i = 0
            while i < len(insts):
                ins = insts[i]
                if (
                    ins.sync_info is not None
                    and len(ins.sync_info.on_wait) > max_waits
                ):
                    waits = list(ins.sync_info.on_wait)
                    keep, extra = waits[:max_waits], waits[max_waits:]
                    ins.sync_info.on_wait = keep
                    for j, wchunk in enumerate(extra):
                        nop = mybir.InstNoOp(
                            name=f"{ins.name}-wsplit{j}",
                            sync_info=mybir.SyncInfo(on_wait=[wchunk], on_update=[]),
                            bass_nofuse=True,
                            engine=ins.engine,
                        )
                        nc.register_instruction(nop, overwrite=True)
                        insts.insert(i, nop)
                        i += 1
                i += 1


def _get_nc():
    global _NC
    if _NC is None:
        _NC = _build_nc()
        _split_ctrl_waits(_NC)
    return _NC


# z = 128*phi + c = 49*m + k for z in [0, 6272)
_Z = np.arange(ROWS * K)
_MZ = _Z // K
_KZ = _Z % K


def _pack_inputs(enc_x, weight, bias):
    """Host-side repack: dense phase-packed bf16 layout + banded stationaries."""
    # xr[m, g, k] = enc_x_core[m*J + g, k]; phase view:
    # X4[phi, c, g] = xr[mz, g, kz] at z = 128*phi + c
    xr = np.asarray(enc_x, dtype=np.float32).reshape(NCORES, ROWS, J, K)
    xb = xr.astype(BF16_NP)
    xT = np.empty((NCORES, ROWS * NPHI * J), dtype=BF16_NP)
    for cix in range(NCORES):
        g1 = xb[cix][_MZ, :, _KZ]                      # [6272, J]
        x4 = g1.reshape(NPHI, ROWS, J)                 # [phi, c, g]
        slabs, g0 = [], 0
        for gw in BLOCKS:
            slabs.append(
                np.ascontiguousarray(
                    x4[:, :, g0 : g0 + gw].transpose(1, 0, 2)
                ).reshape(-1)
            )
            g0 += gw
        xT[cix] = np.concatenate(slabs)

    wb = np.asarray(weight, dtype=np.float32).reshape(K).astype(BF16_NP)
    stat = np.zeros((ROWS, NPHI * ROWS), dtype=BF16_NP)
    for phi in range(NPHI):
        z = 128 * phi + np.arange(ROWS)                # z for each c
        m = z // K
        k = z % K
        valid = m < ROWS
        stat[np.arange(ROWS)[valid], phi * ROWS + m[valid]] = wb[k[valid]]

    bf = np.ascontiguousarray(np.asarray(bias), dtype=np.float32).reshape(1)
    return xT, stat, bf


def run(enc_x, weight, bias, trace=False, **spmd_kwargs):
    """Run on 8 NeuronCores; returns (out [B, WINDOWS] fp32, BassKernelResults)."""
    from concourse.bass_utils import run_bass_kernel_spmd

    nc = _get_nc()
    xT, stat, bf = _pack_inputs(enc_x, weight, bias)
    in_maps = [{"x": xT[i], "s": stat, "b": bf} for i in range(NCORES)]
    res = run_bass_kernel_spmd(
        nc, in_maps, list(range(NCORES)), trace=trace, **spmd_kwargs
    )
    out = np.stack([res.results[i]["out"] for i in range(NCORES)], axis=0)
    return out.reshape(B, WINDOWS), res


def kernel(enc_x, weight, bias, windows_nb=None):
    out, _ = run(enc_x, weight, bias)
    return out


# revision 22
# speedup vs baseline: 1.2343x; 1.0737x over previous
"""Trainium2 Bass kernel for im2col Conv2d dot-product:
out[b, n] = <enc_x[b, n, :], w_flat> + bias.

Data-parallel over batch: 8 batches per NeuronCore x 8 cores.
Per core: x is [401408, 49] -> out [401408] fp32.

v5: dense phase-packed TensorE matmul.

DMA on TRN2 runs at full rate (~427 GB/s, 27 GB/s x 16 engines)
ONLY for 128-partition tiles (measured: 98 partitions -> ~260 GB/s,
112 -> ~263, 120 -> ~199). So the moving operand must fill all 128
partitions with real data, but windows are 49 long: instead of
2-windows-per-column (98 rows + 30 pad), pack the im2col stream
DENSELY: column c of "phase" phi holds flat element 128*phi + c of a
6272-element group (= 128 windows x 49 = lcm(49,128) structure).
The identity 128*phi + c = 49*m + k uniquely assigns every (phi, c)
to window-row m, element k, so 49 banded stationaries
S_phi[c, m] = w[128*phi + c - 49*m] (in [0,49)) make the 49
phase-matmuls accumulate exact per-window dot products into
PSUM[m, g] with zero padding and zero redundancy.

Stream order puts window m*3136+g at stream slot 128*g+m, so
PSUM[m, g-chunk] lands in natural window order: out DMA is
contiguous per partition.  Each 448-column block closes after its
49 matmuls -> ScalarE bias-add -> out DMA, fully pipelined (no
end-of-kernel PSUM flush tail).

Per core: DMA in 39.3 MB bf16 (~95-105 us at full rate), TensorE
153664 moving columns ~ 64 us @2.4 GHz, ScalarE 3.6 us, DVE idle.
"""

from contextlib import ExitStack

import numpy as np
import ml_dtypes

import concourse.bass as bass
import concourse.tile as tile
from concourse import mybir

B = 64
WINDOWS = 50176
K = 49
NCORES = 8
BPC = B // NCORES            # batches per core
NWIN = BPC * WINDOWS         # 401408 windows per core
ROWS = 128                   # window-row grid: window = m*J + g
J = NWIN // ROWS             # 3136
NPHI = K                     # 49 phases
GW = 448                     # g-columns per full block
# last full block split in two: shorter tail after the final x-DMA
BLOCKS = [448] * 6 + [224, 224]          # g-cols per block, sum = J
BATCHES = [(0, 3), (3, 3), (6, 2)]       # out-DMA batching (start, nblocks)
NBLK = len(BLOCKS)

FP32 = mybir.dt.float32
BF16 = mybir.dt.bfloat16
BF16_NP = ml_dtypes.bfloat16

_NC = None


def _build_nc():
    nc = bass.Bass(trn_type="TRN2", debug=False, num_devices=NCORES)

    # x: concatenated per-block [ROWS, NPHI*gw_b] slabs (column phi*gw_b+gw)
    x = nc.dram_tensor("x", [ROWS * NPHI * J], BF16, kind="ExternalInput").ap()
    # compact stationary band: per phase, the <=4 nonzero columns
    s = nc.dram_tensor("s", [ROWS, NPHI * 4], BF16, kind="ExternalInput").ap()
    b = nc.dram_tensor("b", [1], FP32, kind="ExternalInput").ap()
    out = nc.dram_tensor("out", [NWIN], FP32, kind="ExternalOutput").ap()

    with tile.TileContext(nc) as tc, ExitStack() as ctx:
        consts = ctx.enter_context(tc.tile_pool(name="consts", bufs=1))
        xpool = ctx.enter_context(tc.tile_pool(name="x", bufs=3))
        opool = ctx.enter_context(tc.tile_pool(name="o", bufs=2))
        ppool = ctx.enter_context(tc.tile_pool(name="psum", bufs=4, space="PSUM"))

        # stat is mostly zeros (<=4 nonzero cols per phase): DMA only the
        # 100 KB compact band and scatter with the otherwise-idle DVE, so
        # the full 1.6 MB never contends with the x stream (it previously
        # landed ~30 us in, starving the PE's start).
        sc = consts.tile([ROWS, NPHI * 4], BF16)
        nc.gpsimd.dma_start(out=sc[:], in_=s)
        bb = consts.tile([ROWS, 1], FP32)
        nc.gpsimd.dma_start(
            out=bb[:],
            in_=bass.AP(tensor=b.tensor, offset=b.offset, ap=[[0, ROWS]] + list(b.ap)),
        )
        stat = consts.tile([ROWS, NPHI * ROWS], BF16)
        nc.vector.memset(stat[:], 0.0)
        for phi in range(NPHI):
            m_lo = (128 * phi) // K
            width = min(4, ROWS - m_lo)
            nc.vector.tensor_copy(
                out=stat[:, phi * ROWS + m_lo : phi * ROWS + m_lo + width],
                in_=sc[:, phi * 4 : phi * 4 + width],
            )

        blk_g0 = [sum(BLOCKS[:i]) for i in range(NBLK)]   # g-offset per block
        xoff = 0
        for bstart, bn in BATCHES:
            bcols = sum(BLOCKS[bstart : bstart + bn])
            obuf = opool.tile([ROWS, bcols], FP32, tag="ot", name=f"ot{bstart}")
            ocol = 0
            for blk in range(bstart, bstart + bn):
                gw = BLOCKS[blk]
                xt = xpool.tile([ROWS, NPHI * gw], BF16, tag="xt", name=f"xt{blk}")
                src = bass.AP(
                    tensor=x.tensor,
                    offset=x.offset + xoff,
                    ap=[[NPHI * gw, ROWS], [1, NPHI * gw]],
                )
                xoff += ROWS * NPHI * gw
                nc.sync.dma_start(out=xt[:], in_=src)

                # full PSUM bank (512 fp32) for alignment; use gw cols
                acc = ppool.tile([ROWS, 512], FP32, tag="acc", name=f"acc{blk}")
                for phi in range(NPHI):
                    nc.tensor.matmul(
                        acc[:, 0:gw],
                        stat[:, phi * ROWS : (phi + 1) * ROWS],
                        xt[:, phi * gw : (phi + 1) * gw],
                        start=(phi == 0),
                        stop=(phi == NPHI - 1),
                    )

                nc.scalar.activation(
                    out=obuf[:, ocol : ocol + gw],
                    in_=acc[:, 0:gw],
                    func=mybir.ActivationFunctionType.Identity,
                    bias=bb[:, 0:1],
                    scale=1.0,
                )
                ocol += gw

            dst = bass.AP(
                tensor=out.tensor,
                offset=out.offset + blk_g0[bstart],
                ap=[[J, ROWS], [1, bcols]],
            )
            # final batch rides the (by-then idle) HWDGE sync queue
            oeng = nc.sync if bstart + bn == NBLK else nc.gpsimd
            oeng.dma_start(out=dst, in_=obuf[:])

    return nc


def _split_ctrl_waits(nc, max_waits=1):
    """Work around a walrus codegen limit on this build: instructions accept
    only one sync-wait command. Hoist extra waits onto dedicated no-op
    instructions inserted just before, preserving per-engine order."""
    from concourse import mybir

    for f in nc.m.functions:
        for blk in f.blocks:
            insts = blk.instructions
            i = 0
            while i < len(insts):
                ins = insts[i]
                if (
                    ins.sync_info is not None
                    and len(ins.sync_info.on_wait) > max_waits
                ):
                    waits = list(ins.sync_info.on_wait)
                    keep, extra = waits[:max_waits], waits[max_waits:]
                    ins.sync_info.on_wait = keep
                    for j, wchunk in enumerate(extra):
                        nop = mybir.InstNoOp(
                            name=f"{ins.name}-wsplit{j}",
                            sync_info=mybir.SyncInfo(on_wait=[wchunk], on_update=[]),
                            bass_nofuse=True,
                            engine=ins.engine,
                        )
                        nc.register_instruction(nop, overwrite=True)
                        insts.insert(i, nop)
                        i += 1
                i += 1


def _get_nc():
    global _NC
    if _NC is None:
        _NC = _build_nc()
        _split_ctrl_waits(_NC)
    return _NC


# z = 128*phi + c = 49*m + k for z in [0, 6272)
_Z = np.arange(ROWS * K)
_MZ = _Z // K
_KZ = _Z % K


def _pack_inputs(enc_x, weight, bias):
    """Host-side repack: dense phase-packed bf16 layout + banded stationaries."""
    # xr[m, g, k] = enc_x_core[m*J + g, k]; phase view:
    # X4[phi, c, g] = xr[mz, g, kz] at z = 128*phi + c
    xr = np.asarray(enc_x, dtype=np.float32).reshape(NCORES, ROWS, J, K)
    xb = xr.astype(BF16_NP)
    xT = np.empty((NCORES, ROWS * NPHI * J), dtype=BF16_NP)
    for cix in range(NCORES):
        g1 = xb[cix][_MZ, :, _KZ]                      # [6272, J]
        x4 = g1.reshape(NPHI, ROWS, J)                 # [phi, c, g]
        slabs, g0 = [], 0
        for gw in BLOCKS:
            slabs.append(
                np.ascontiguousarray(
                    x4[:, :, g0 : g0 + gw].transpose(1, 0, 2)
                ).reshape(-1)
            )
            g0 += gw
        xT[cix] = np.concatenate(slabs)

    wb = np.asarray(weight, dtype=np.float32).reshape(K).astype(BF16_NP)
    # compact band: col phi*4 + (m - m_lo(phi)) holds stat col phi*128 + m
    stat = np.zeros((ROWS, NPHI * 4), dtype=BF16_NP)
    for phi in range(NPHI):
        z = 128 * phi + np.arange(ROWS)                # z for each c
        m = z // K
        k = z % K
        m_lo = (128 * phi) // K
        stat[np.arange(ROWS), phi * 4 + (m - m_lo)] = wb[k]

    bf = np.ascontiguousarray(np.asarray(bias), dtype=np.float32).reshape(1)
    return xT, stat, bf


def run(enc_x, weight, bias, trace=False, **spmd_kwargs):
    """Run on 8 NeuronCores; returns (out [B, WINDOWS] fp32, BassKernelResults)."""
    from concourse.bass_utils import run_bass_kernel_spmd

    nc = _get_nc()
    xT, stat, bf = _pack_inputs(enc_x, weight, bias)
    in_maps = [{"x": xT[i], "s": stat, "b": bf} for i in range(NCORES)]
    res = run_bass_kernel_spmd(
        nc, in_maps, list(range(NCORES)), trace=trace, **spmd_kwargs
    )
    out = np.stack([res.results[i]["out"] for i in range(NCORES)], axis=0)
    return out.reshape(B, WINDOWS), res


def kernel(enc_x, weight, bias, windows_nb=None):
    out, _ = run(enc_x, weight, bias)
    return out
